# revision 4
# baseline (speedup 1.0000x reference)
"""Trainium2 Bass kernel for nn_Correlation: -mean(einsum('itj,itl->ijl', x, y)).

Math: mean over [B, C, C] of corr[b,j,l] = sum_t x[b,t,j] y[b,t,l] equals
  (1/(B*C^2)) * sum_{b,t} (sum_j x[b,t,j]) * (sum_l y[b,t,l])
so the kernel only needs per-row sums of x and y plus a dot product —
a pure memory-bound streaming reduction (no matmul).

Sharding: data-parallel over batch. 8 cores, 1 batch element each.

Schedule (from trace analysis of the previous build): the core's DMA
fabric plateaus at ~434 GB/s shared across the two HWDGE rings; each
DGE channel round-robins one descriptor per queue, so equal descriptor
sizes on the two rings split bandwidth 50/50 and both streams finish
together. x streams on the SP ring, y on the ACT ring, with IDENTICAL
chunk layouts [6,4,3,2,1] rows/partition so descriptor sizes pair up.
Fine-grained chunks keep both consumers (DVE tensor_reduce for x-row
sums, ACT activation-accumulate for y-row sums) running during the
stream instead of idling until a huge chunk completes (the old
11-row y chunk only completed at t=30.7us, pushing 22us of ACT work
into the tail). The final 1-row chunks make the post-stream reduce
tail ~1.4us.

The dot product now happens ON DEVICE: one DVE tensor_tensor_reduce
multiplies the x/y row-sum tiles and row-reduces to a [128,1] result,
which a single SWDGE store hands to the host (sums 128 values/core).
This removes one store+drain pair from the tail.

Constraints honored (this walrus build allows ONE sync wait per
instruction):
- every chunk gets a dedicated SBUF slot (no WAR/WAW waits on loads);
- activation writes in place (a scratch tile's WAW reuse would add a
  second wait);
- 10 HWDGE loads reuse completion lanes DMAHW0-1 — lane reuse only
  raises consumer wait thresholds (sems accumulate), the triggers
  themselves stay wait-free;
- the single store waits only on DVE; the tail drain waits only on the
  store's SWDGE lane (its completion transitively implies every load
  lane was consumed).
"""

import numpy as np

B, T, C = 8, 2048, 1024
P = 128             # SBUF partitions
RPP = T // P        # rows per partition (16)
# rows/partition per chunk (sums to RPP), identical for x and y so the
# per-descriptor round-robin across the two HWDGE rings splits 50/50.
# Descending sizes: big chunks amortize trigger cost early, the 1-row
# final chunks keep the post-stream reduce tail short.
CHUNKS = [6, 4, 3, 2, 1]
N_CORES = 8

_CACHE = {}


def _patch_tail_drain(tile):
    """Split TileContext's kernel-tail drain into one drain per proc lane.

    The stock tail emits a single SP Drain waiting on every outstanding
    sem (DVE + ACT + each DMA completion lane); this walrus build caps
    sync waits per instruction below that, so codegen fails with "Too
    many sync wait commands". Waiting on the sems one drain at a time is
    equivalent (SP program order) and keeps every instruction at 1 wait.

    Minimal closure for THIS kernel: the single SWDGE store (DMASW0,
    proc 11) waited on DVE; DVE's final tensor_tensor_reduce waited on
    ACT; DVE/ACT waited on every load lane. So draining the store lane
    alone covers everything. Fall back to draining every nonzero lane
    if the tick pattern is unexpected.
    """
    import re
    import bass_rust
    from concourse.vector_clock import ScopedClock

    if getattr(tile.TileContext, "_tail_drain_split", False):
        return

    def _drain_and_barrier(self, tick_clock, wait_clock):
        ticks = [int(s) for s in re.findall(r"-?\d+",
                                            repr(tick_clock.global_clock))]
        swdge0 = 11  # DMASW0 proc lane
        n_loads = 2 * len(CHUNKS)
        hw_ticks = ticks[19:27]  # DMAHW0..7
        expect_hw = [(n_loads + 7 - i) // 8 for i in range(8)]
        if (0 <= swdge0 < len(ticks) and ticks[swdge0] == 1
                and hw_ticks == expect_hw):
            lanes = [swdge0]
        else:
            lanes = [i for i, t in reversed(list(enumerate(ticks))) if t > 0]
        for i in lanes:
            part = bass_rust.VectorClock(
                [ticks[i] if j == i else 0 for j in range(len(ticks))])
            d = self.nc.sync.drain()
            wait_clock.add_sem_waits(d.ins, ScopedClock({None: part}))
        self.nc.all_engine_barrier()
        assert self.sems is not None
        popped = self.nc._tile_sem_poison_stack.pop()
        assert popped is self._sem_poison
        # no second barrier: the NRT postamble's full sem sweep makes any
        # clear-vs-postamble write race benign (both write zero)
        self.nc.clear_and_free_semaphores(list(self.sems.allocated().values()))

    tile.TileContext._drain_and_barrier = _drain_and_barrier
    tile.TileContext._tail_drain_split = True


def _build_bass():
    import concourse.bass as bass
    import concourse.tile as tile
    from concourse import mybir

    _patch_tail_drain(tile)

    f32 = mybir.dt.float32
    # Bass.__init__ unconditionally memsets a const pool and emits an
    # all-engine barrier (~0.7 us on the measured critical path). This
    # kernel never reads the const APs, so suppress both during init.
    _ob, _om = bass.Bass.all_engine_barrier, bass.BassSharedVectorInterface.memset
    bass.Bass.all_engine_barrier = lambda self, *a, **k: None
    bass.BassSharedVectorInterface.memset = lambda self, *a, **k: None
    try:
        nc = bass.Bass()
    finally:
        bass.Bass.all_engine_barrier = _ob
        bass.BassSharedVectorInterface.memset = _om
    x = nc.dram_tensor("x", [T, C], f32, kind="ExternalInput")
    y = nc.dram_tensor("y", [T, C], f32, kind="ExternalInput")
    out = nc.dram_tensor("out", [P, 1], f32, kind="ExternalOutput")

    with tile.TileContext(nc) as tc:
        with (
            # dedicated slot per chunk (unique tags, 1 buf each): load DMAs
            # never carry WAR/WAW waits
            tc.tile_pool(name="iox", bufs=1) as iox,
            tc.tile_pool(name="ioy", bufs=1) as ioy,
            tc.tile_pool(name="acc", bufs=1) as acc,
        ):
            sxy = acc.tile([P, 2, RPP], f32)  # [:,0,:] x sums, [:,1,:] y sums
            prod = acc.tile([P, RPP], f32)
            dummy = acc.tile([P, 1], f32)
            res = acc.tile([P, 1], f32)

            # all load triggers first: x on the SP ring, y on the ACT ring.
            # Interleaved issue keeps both descriptor queues fed from the
            # first microsecond; the y triggers sit ahead of the slow
            # activations in ACT program order.
            xts, yts = [], []
            off = 0
            for a in CHUNKS:
                yt = ioy.tile([P, a, C], f32, tag=f"yt{off}")
                nc.scalar.dma_start(
                    out=yt[:],
                    in_=y[off * P:(off + a) * P, :]
                        .rearrange("(p a) c -> p a c", p=P))
                yts.append((off, a, yt))
                xt = iox.tile([P, a, C], f32, tag=f"xt{off}")
                nc.sync.dma_start(
                    out=xt[:],
                    in_=x[off * P:(off + a) * P, :]
                        .rearrange("(p a) c -> p a c", p=P))
                xts.append((off, a, xt))
                off += a

            for off, a, xt in xts:
                nc.vector.tensor_reduce(
                    out=sxy[:, 0, off:off + a], in_=xt[:],
                    axis=mybir.AxisListType.X, op=mybir.AluOpType.add,
                )
            for off, a, yt in yts:
                for j in range(a):
                    nc.scalar.activation(
                        out=yt[:, j], in_=yt[:, j],
                        func=mybir.ActivationFunctionType.Copy,
                        accum_out=sxy[:, 1, off + j:off + j + 1],
                    )

            # per-partition dot product res[p] = sum_j sx[p,j]*sy[p,j].
            # TensorTensor's ISA struct in this walrus build has ZERO sync
            # wait slots (and TensorTensorReduce mis-encodes entirely), so a
            # cheap tensor_reduce "wait carrier" takes the 1 wait on ACT's
            # lane first; the TT + final reduce then run wait-free in DVE
            # program order.
            nc.vector.tensor_reduce(
                out=dummy[:], in_=sxy[:, 1, :],
                axis=mybir.AxisListType.X, op=mybir.AluOpType.add,
            )
            nc.vector.tensor_tensor(
                out=prod[:], in0=sxy[:, 0, :], in1=sxy[:, 1, :],
                op=mybir.AluOpType.mult,
            )
            nc.vector.tensor_reduce(
                out=res[:], in_=prod[:],
                axis=mybir.AxisListType.X, op=mybir.AluOpType.add,
            )

            # single tiny store on the (fresh) SWDGE lane; waits only on DVE
            nc.gpsimd.dma_start(out=out[:], in_=res[:])
    return nc


def _run(x, y, trace=False):
    from concourse.bass_utils import run_bass_kernel_spmd

    if "nc" not in _CACHE:
        _CACHE["nc"] = _build_bass()
    nc = _CACHE["nc"]
    in_maps = [
        {"x": np.ascontiguousarray(x[i]), "y": np.ascontiguousarray(y[i])}
        for i in range(N_CORES)
    ]
    return run_bass_kernel_spmd(nc, in_maps, core_ids=list(range(N_CORES)),
                                trace=trace)


def kernel(**inputs) -> np.ndarray:
    x = np.asarray(inputs["x"], dtype=np.float32)
    y = np.asarray(inputs["y"], dtype=np.float32)
    res = _run(x, y, trace=False)
    s = 0.0
    for r in res.results:
        s += r["out"].astype(np.float64).sum()
    return np.array(-s / (B * C * C), dtype=np.float32)


# revision 19
# speedup vs baseline: 1.0909x; 1.0909x over previous
"""Trainium2 Bass kernel for nn_Correlation: -mean(einsum('itj,itl->ijl', x, y)).

Math: mean over [B, C, C] of corr[b,j,l] = sum_t x[b,t,j] y[b,t,l] equals
  (1/(B*C^2)) * sum_{b,t} (sum_j x[b,t,j]) * (sum_l y[b,t,l])
so the kernel only needs per-row sums of x and y plus a dot product —
a pure memory-bound streaming reduction (no matmul).

Sharding: data-parallel over batch. 8 cores, 1 batch element each.

Schedule (from trace analysis): the core's DMA fabric plateaus at
~434 GB/s shared by the two HWDGE rings; each of the 16 DGE channels
round-robins one descriptor per queue at a ~27 GB/s per-channel
ceiling. x streams on the SP ring, y on the ACT ring. Chunk layouts
are deliberately STAGGERED ([7,4,3,1,1] vs [6,5,2,2,1] rows/partition)
— with identical layouts every channel alternates two descriptors
exactly 8 MB apart in HBM and channel 15 loses arbitration ~20%,
lagging 9.5 us behind and gating every chunk-completion semaphore.
Fine-grained chunks keep both consumers (DVE tensor_reduce for x-row
sums, ACT activation-accumulate for y rows 1..15) working during the
stream; the 1-row final chunks keep the post-stream tail short. The
last y row is summed on DVE (1.07 us/row vs ACT's 1.41) right after
its own last x chunk.

Because the two layouts place a given row at different tile columns,
the dot product stays on the HOST (order-independent after un-permute);
the row-sum tile is laid out [P, 33] — cols 0-15 x sums + col 16 the
DVE y tail (all DVE-written, contiguous), cols 17-31 ACT's y sums — so
TWO stores suffice: one SWDGE store for the DVE half (1 DVE wait, lane
DMASW0 fresh) and one ACT-ring store for ACT's half (ACT program
order; its only wait is the completion-lane reuse wait).

Constraints honored (this walrus build allows ONE sync wait per
instruction; TensorTensor allows ZERO, and TensorTensorReduce /
scalar_tensor_tensor mis-encode entirely):
- every chunk gets a dedicated SBUF slot (no WAR/WAW waits on loads);
- activation writes in place (a scratch tile's WAW reuse would add a
  second wait);
- HWDGE completion-lane reuse adds a WAR wait to the TRIGGER (verified
  empirically), so the two 1-row tail loads (lanes DMAHW0-1 reused)
  carry exactly that one wait — their triggers stall until the first
  chunks complete (~22 us), harmless since the consumers wait longer —
  and any store with a data wait must use a fresh SWDGE lane;
- the tail drain waits only on the two store lanes (their completion
  transitively implies every load lane was consumed).
"""

import numpy as np

B, T, C = 8, 2048, 1024
P = 128             # SBUF partitions
RPP = T // P        # rows per partition (16)
# rows/partition per chunk (each sums to RPP). Staggered sizes between
# the rings (see module docstring); descending so the final chunks are
# 1 row. ACT consumes y chunks 0..3 (15 rows); DVE consumes all x
# chunks plus the last y row.
XCHUNKS = [7, 4, 3, 1, 1]
YCHUNKS = [6, 5, 2, 2, 1]
N_CORES = 8

_CACHE = {}


def _patch_tail_drain(tile):
    """Split TileContext's kernel-tail drain into one drain per proc lane.

    The stock tail emits a single SP Drain waiting on every outstanding
    sem (DVE + ACT + each DMA completion lane); this walrus build caps
    sync waits per instruction below that, so codegen fails with "Too
    many sync wait commands". Waiting on the sems one drain at a time is
    equivalent (SP program order) and keeps every instruction at 1 wait.

    Minimal closure for THIS kernel: the SWDGE store (DMASW0) waited on
    DVE which waited on every x lane + the y-tail lane; the ACT-ring
    store (11th HWDGE DMA, lane DMAHW2 tick 2) follows ACT's
    activations which waited on the other y lanes. Draining those two
    lanes' full ticks covers everything. Fall back to draining every
    nonzero lane if the tick pattern is unexpected.
    """
    import re
    import bass_rust
    from concourse.vector_clock import ScopedClock

    if getattr(tile.TileContext, "_tail_drain_split", False):
        return

    def _drain_and_barrier(self, tick_clock, wait_clock):
        ticks = [int(s) for s in re.findall(r"-?\d+",
                                            repr(tick_clock.global_clock))]
        n_loads = len(XCHUNKS) + len(YCHUNKS)
        n_hw = n_loads + 1  # loads + ACT store
        expect_hw = [(n_hw + 7 - i) // 8 for i in range(8)]
        act_store_lane = 19 + n_loads % 8
        if (len(ticks) >= 27 and ticks[19:27] == expect_hw
                and ticks[11] == 1 and all(t == 0 for t in ticks[12:19])):
            lanes = [11, act_store_lane]  # DMASW0 + ACT store lane
        else:
            lanes = [i for i, t in reversed(list(enumerate(ticks))) if t > 0]
        for i in lanes:
            part = bass_rust.VectorClock(
                [ticks[i] if j == i else 0 for j in range(len(ticks))])
            d = self.nc.sync.drain()
            wait_clock.add_sem_waits(d.ins, ScopedClock({None: part}))
        self.nc.all_engine_barrier()
        assert self.sems is not None
        popped = self.nc._tile_sem_poison_stack.pop()
        assert popped is self._sem_poison
        # no second barrier: the NRT postamble's full sem sweep makes any
        # clear-vs-postamble write race benign (both write zero)
        self.nc.clear_and_free_semaphores(list(self.sems.allocated().values()))

    tile.TileContext._drain_and_barrier = _drain_and_barrier
    tile.TileContext._tail_drain_split = True


def _build_bass():
    import concourse.bass as bass
    import concourse.tile as tile
    from concourse import mybir

    _patch_tail_drain(tile)

    f32 = mybir.dt.float32
    # Bass.__init__ unconditionally memsets a const pool and emits an
    # all-engine barrier (~0.7 us on the measured critical path). This
    # kernel never reads the const APs, so suppress both during init.
    _ob, _om = bass.Bass.all_engine_barrier, bass.BassSharedVectorInterface.memset
    bass.Bass.all_engine_barrier = lambda self, *a, **k: None
    bass.BassSharedVectorInterface.memset = lambda self, *a, **k: None
    try:
        nc = bass.Bass()
    finally:
        bass.Bass.all_engine_barrier = _ob
        bass.BassSharedVectorInterface.memset = _om
    x = nc.dram_tensor("x", [T, C], f32, kind="ExternalInput")
    y = nc.dram_tensor("y", [T, C], f32, kind="ExternalInput")
    out = nc.dram_tensor("out", [P, 2 * RPP], f32, kind="ExternalOutput")

    with tile.TileContext(nc) as tc:
        with (
            # dedicated slot per chunk (unique tags, 1 buf each): load DMAs
            # never carry WAR/WAW waits
            tc.tile_pool(name="iox", bufs=1) as iox,
            tc.tile_pool(name="ioy", bufs=1) as ioy,
            tc.tile_pool(name="acc", bufs=1) as acc,
        ):
            # cols 0-15: x sums; col 16: DVE's y tail row; cols 17-31:
            # ACT's y sums (rows 0-14). DVE's outputs are contiguous so
            # one store covers them.
            sxy = acc.tile([P, 2 * RPP], f32)

            # all load triggers first: x on the SP ring, y on the ACT ring.
            # Interleaved issue keeps both descriptor queues fed from the
            # first microsecond; the y triggers sit ahead of the slow
            # activations in ACT program order.
            xts, yts = [], []
            offx = offy = 0
            for i in range(max(len(XCHUNKS), len(YCHUNKS))):
                if i < len(YCHUNKS):
                    a = YCHUNKS[i]
                    yt = ioy.tile([P, a, C], f32, tag=f"yt{offy}")
                    nc.scalar.dma_start(
                        out=yt[:],
                        in_=y[offy * P:(offy + a) * P, :]
                            .rearrange("(p a) c -> p a c", p=P))
                    yts.append((offy, a, yt))
                    offy += a
                if i < len(XCHUNKS):
                    a = XCHUNKS[i]
                    xt = iox.tile([P, a, C], f32, tag=f"xt{offx}")
                    nc.sync.dma_start(
                        out=xt[:],
                        in_=x[offx * P:(offx + a) * P, :]
                            .rearrange("(p a) c -> p a c", p=P))
                    xts.append((offx, a, xt))
                    offx += a

            # DVE: x row sums per chunk (1 wait each on the chunk's lane)
            for off, a, xt in xts:
                nc.vector.tensor_reduce(
                    out=sxy[:, off:off + a], in_=xt[:],
                    axis=mybir.AxisListType.X, op=mybir.AluOpType.add,
                )
            # ACT: y row sums for chunks 0..n-2 (rows 0..14), in place
            for off, a, yt in yts[:-1]:
                for j in range(a):
                    nc.scalar.activation(
                        out=yt[:, j], in_=yt[:, j],
                        func=mybir.ActivationFunctionType.Copy,
                        accum_out=sxy[:, RPP + 1 + off + j:RPP + 2 + off + j],
                    )
            # ACT's y sums ride the ACT ring in ACT program order (its
            # only wait is the lane-reuse WAR). Created BEFORE the DVE
            # tail reduce so the tracker cannot attach a DVE wait.
            nc.scalar.dma_start(out=out[:, RPP + 1:], in_=sxy[:, RPP + 1:])

            # DVE: last y row (faster per row than ACT; lands last)
            offl, al, ytl = yts[-1]
            nc.vector.tensor_reduce(
                out=sxy[:, RPP:RPP + 1], in_=ytl[:],
                axis=mybir.AxisListType.X, op=mybir.AluOpType.add,
            )

            # DVE's half (x sums + y tail, contiguous) on the fresh
            # SWDGE lane: one DVE wait
            nc.gpsimd.dma_start(out=out[:, :RPP + 1], in_=sxy[:, :RPP + 1])
    return nc


def _run(x, y, trace=False):
    from concourse.bass_utils import run_bass_kernel_spmd

    if "nc" not in _CACHE:
        _CACHE["nc"] = _build_bass()
    nc = _CACHE["nc"]
    in_maps = [
        {"x": np.ascontiguousarray(x[i]), "y": np.ascontiguousarray(y[i])}
        for i in range(N_CORES)
    ]
    return run_bass_kernel_spmd(nc, in_maps, core_ids=list(range(N_CORES)),
                                trace=trace)


def _row_map(chunks):
    """row index for each (partition, column) of the on-chip sum tile:
    chunk at column offset `off` with `a` rows/partition holds row
    off*P + p*a + j in column off+j."""
    m = np.empty((P, RPP), np.int64)
    off = 0
    for a in chunks:
        for j in range(a):
            m[:, off + j] = off * P + np.arange(P) * a + j
        off += a
    return m


_XMAP = _row_map(XCHUNKS)
_YMAP = _row_map(YCHUNKS)


def kernel(**inputs) -> np.ndarray:
    x = np.asarray(inputs["x"], dtype=np.float32)
    y = np.asarray(inputs["y"], dtype=np.float32)
    res = _run(x, y, trace=False)
    s = 0.0
    for r in res.results:
        o = r["out"].astype(np.float64)
        sx = np.empty(T); sx[_XMAP.ravel()] = o[:, :RPP].ravel()
        # device col 16 = y rows per YMAP col 15; cols 17.. = YMAP cols 0-14
        sy = np.empty(T)
        sy[_YMAP[:, :RPP - 1].ravel()] = o[:, RPP + 1:].ravel()
        sy[_YMAP[:, RPP - 1]] = o[:, RPP]
        s += (sx * sy).sum()
    return np.array(-s / (B * C * C), dtype=np.float32)


# revision 20
# speedup vs baseline: 1.2194x; 1.1177x over previous
"""Trainium2 Bass kernel for nn_Correlation: -mean(einsum('itj,itl->ijl', x, y)).

Math: mean over [B, C, C] of corr[b,j,l] = sum_t x[b,t,j] y[b,t,l] equals
  (1/(B*C^2)) * sum_{b,t} (sum_j x[b,t,j]) * (sum_l y[b,t,l])
so the kernel only needs per-row sums of x and y plus a dot product —
a pure memory-bound streaming reduction (no matmul).

Sharding: data-parallel over batch. 8 cores, 1 batch element each.

Schedule (from trace analysis): the core's DMA fabric plateaus at
~434 GB/s shared by the two HWDGE rings; each of the 16 DGE channels
round-robins one descriptor per queue at a ~27 GB/s per-channel
ceiling. x streams on the SP ring, y on the ACT ring. Chunk layouts
are deliberately STAGGERED ([7,4,3,1,1] vs [6,5,2,2,1] rows/partition)
— with identical layouts every channel alternates two descriptors
exactly 8 MB apart in HBM and channel 15 loses arbitration ~20%,
lagging 9.5 us behind and gating every chunk-completion semaphore.
Fine-grained chunks keep both consumers (DVE tensor_reduce for x-row
sums, ACT activation-accumulate for y rows 1..15) working during the
stream; the 1-row final chunks keep the post-stream tail short. The
last y row is summed on DVE (1.07 us/row vs ACT's 1.41) right after
its own last x chunk.

Because the two layouts place a given row at different tile columns,
the dot product stays on the HOST (order-independent after un-permute);
the row-sum tile is laid out [P, 33] — cols 0-15 x sums + col 16 the
DVE y tail (all DVE-written, contiguous), cols 17-31 ACT's y sums — so
TWO stores suffice: one SWDGE store for the DVE half (1 DVE wait, lane
DMASW0 fresh) and one ACT-ring store for ACT's half (ACT program
order; its only wait is the completion-lane reuse wait).

Constraints honored (this walrus build allows ONE sync wait per
instruction; TensorTensor allows ZERO, and TensorTensorReduce /
scalar_tensor_tensor mis-encode entirely):
- every chunk gets a dedicated SBUF slot (no WAR/WAW waits on loads);
- activation writes in place (a scratch tile's WAW reuse would add a
  second wait);
- HWDGE completion-lane reuse adds a WAR wait to the TRIGGER (verified
  empirically), so the two 1-row tail loads (lanes DMAHW0-1 reused)
  carry exactly that one wait — their triggers stall until the first
  chunks complete (~22 us), harmless since the consumers wait longer —
  and any store with a data wait must use a fresh SWDGE lane;
- the tail drain waits only on the two store lanes (their completion
  transitively implies every load lane was consumed).
"""

import numpy as np

B, T, C = 8, 2048, 1024
P = 128             # SBUF partitions
RPP = T // P        # rows per partition (16)
# rows/partition per chunk (each sums to RPP). Staggered sizes between
# the rings (see module docstring); descending so the final chunks are
# 1 row. ACT consumes y chunks 0..3 (15 rows); DVE consumes all x
# chunks plus the last y row.
XCHUNKS = [7, 4, 3, 2]
YCHUNKS = [6, 5, 4, 1]
N_CORES = 8

_CACHE = {}


def _patch_tail_drain(tile):
    """Split TileContext's kernel-tail drain into one drain per proc lane.

    The stock tail emits a single SP Drain waiting on every outstanding
    sem (DVE + ACT + each DMA completion lane); this walrus build caps
    sync waits per instruction below that, so codegen fails with "Too
    many sync wait commands". Waiting on the sems one drain at a time is
    equivalent (SP program order) and keeps every instruction at 1 wait.

    Minimal closure for THIS kernel: the SWDGE store (DMASW0) waited on
    DVE which waited on every x lane + the y-tail lane; the ACT-ring
    store (11th HWDGE DMA, lane DMAHW2 tick 2) follows ACT's
    activations which waited on the other y lanes. Draining those two
    lanes' full ticks covers everything. Fall back to draining every
    nonzero lane if the tick pattern is unexpected.
    """
    import re
    import bass_rust
    from concourse.vector_clock import ScopedClock

    if getattr(tile.TileContext, "_tail_drain_split", False):
        return

    def _drain_and_barrier(self, tick_clock, wait_clock):
        ticks = [int(s) for s in re.findall(r"-?\d+",
                                            repr(tick_clock.global_clock))]
        n_loads = len(XCHUNKS) + len(YCHUNKS)
        n_hw = n_loads + 1  # loads + ACT store
        expect_hw = [(n_hw + 7 - i) // 8 for i in range(8)]
        act_store_lane = 19 + n_loads % 8
        if (len(ticks) >= 27 and ticks[19:27] == expect_hw
                and ticks[11] == 1 and all(t == 0 for t in ticks[12:19])):
            lanes = [11, act_store_lane]  # DMASW0 + ACT store lane
        else:
            lanes = [i for i, t in reversed(list(enumerate(ticks))) if t > 0]
        for i in lanes:
            part = bass_rust.VectorClock(
                [ticks[i] if j == i else 0 for j in range(len(ticks))])
            d = self.nc.sync.drain()
            wait_clock.add_sem_waits(d.ins, ScopedClock({None: part}))
        self.nc.all_engine_barrier()
        assert self.sems is not None
        popped = self.nc._tile_sem_poison_stack.pop()
        assert popped is self._sem_poison
        # no second barrier: the NRT postamble's full sem sweep makes any
        # clear-vs-postamble write race benign (both write zero)
        self.nc.clear_and_free_semaphores(list(self.sems.allocated().values()))

    tile.TileContext._drain_and_barrier = _drain_and_barrier
    tile.TileContext._tail_drain_split = True


def _build_bass():
    import concourse.bass as bass
    import concourse.tile as tile
    from concourse import mybir

    _patch_tail_drain(tile)

    f32 = mybir.dt.float32
    # Bass.__init__ unconditionally memsets a const pool and emits an
    # all-engine barrier (~0.7 us on the measured critical path). This
    # kernel never reads the const APs, so suppress both during init.
    _ob, _om = bass.Bass.all_engine_barrier, bass.BassSharedVectorInterface.memset
    bass.Bass.all_engine_barrier = lambda self, *a, **k: None
    bass.BassSharedVectorInterface.memset = lambda self, *a, **k: None
    try:
        nc = bass.Bass()
    finally:
        bass.Bass.all_engine_barrier = _ob
        bass.BassSharedVectorInterface.memset = _om
    x = nc.dram_tensor("x", [T, C], f32, kind="ExternalInput")
    y = nc.dram_tensor("y", [T, C], f32, kind="ExternalInput")
    out = nc.dram_tensor("out", [P, 2 * RPP], f32, kind="ExternalOutput")

    with tile.TileContext(nc) as tc:
        with (
            # dedicated slot per chunk (unique tags, 1 buf each): load DMAs
            # never carry WAR/WAW waits
            tc.tile_pool(name="iox", bufs=1) as iox,
            tc.tile_pool(name="ioy", bufs=1) as ioy,
            tc.tile_pool(name="acc", bufs=1) as acc,
        ):
            # cols 0-15: x sums; col 16: DVE's y tail row; cols 17-31:
            # ACT's y sums (rows 0-14). DVE's outputs are contiguous so
            # one store covers them.
            sxy = acc.tile([P, 2 * RPP], f32)

            # all load triggers first: x on the SP ring, y on the ACT ring.
            # Interleaved issue keeps both descriptor queues fed from the
            # first microsecond; the y triggers sit ahead of the slow
            # activations in ACT program order.
            xts, yts = [], []
            offx = offy = 0
            for i in range(max(len(XCHUNKS), len(YCHUNKS))):
                if i < len(YCHUNKS):
                    a = YCHUNKS[i]
                    yt = ioy.tile([P, a, C], f32, tag=f"yt{offy}")
                    nc.scalar.dma_start(
                        out=yt[:],
                        in_=y[offy * P:(offy + a) * P, :]
                            .rearrange("(p a) c -> p a c", p=P))
                    yts.append((offy, a, yt))
                    offy += a
                if i < len(XCHUNKS):
                    a = XCHUNKS[i]
                    xt = iox.tile([P, a, C], f32, tag=f"xt{offx}")
                    nc.sync.dma_start(
                        out=xt[:],
                        in_=x[offx * P:(offx + a) * P, :]
                            .rearrange("(p a) c -> p a c", p=P))
                    xts.append((offx, a, xt))
                    offx += a

            # DVE: x row sums per chunk (1 wait each on the chunk's lane)
            for off, a, xt in xts:
                nc.vector.tensor_reduce(
                    out=sxy[:, off:off + a], in_=xt[:],
                    axis=mybir.AxisListType.X, op=mybir.AluOpType.add,
                )
            # ACT: y row sums for chunks 0..n-2 (rows 0..14), in place
            for off, a, yt in yts[:-1]:
                for j in range(a):
                    nc.scalar.activation(
                        out=yt[:, j], in_=yt[:, j],
                        func=mybir.ActivationFunctionType.Copy,
                        accum_out=sxy[:, RPP + 1 + off + j:RPP + 2 + off + j],
                    )
            # ACT's y sums ride the ACT ring in ACT program order (its
            # only wait is the lane-reuse WAR). Created BEFORE the DVE
            # tail reduce so the tracker cannot attach a DVE wait.
            nc.scalar.dma_start(out=out[:, RPP + 1:], in_=sxy[:, RPP + 1:])

            # DVE: last y row (faster per row than ACT; lands last)
            offl, al, ytl = yts[-1]
            nc.vector.tensor_reduce(
                out=sxy[:, RPP:RPP + 1], in_=ytl[:],
                axis=mybir.AxisListType.X, op=mybir.AluOpType.add,
            )

            # DVE's half (x sums + y tail, contiguous) on the fresh
            # SWDGE lane: one DVE wait
            nc.gpsimd.dma_start(out=out[:, :RPP + 1], in_=sxy[:, :RPP + 1])
    return nc


def _run(x, y, trace=False):
    from concourse.bass_utils import run_bass_kernel_spmd

    if "nc" not in _CACHE:
        _CACHE["nc"] = _build_bass()
    nc = _CACHE["nc"]
    in_maps = [
        {"x": np.ascontiguousarray(x[i]), "y": np.ascontiguousarray(y[i])}
        for i in range(N_CORES)
    ]
    return run_bass_kernel_spmd(nc, in_maps, core_ids=list(range(N_CORES)),
                                trace=trace)


def _row_map(chunks):
    """row index for each (partition, column) of the on-chip sum tile:
    chunk at column offset `off` with `a` rows/partition holds row
    off*P + p*a + j in column off+j."""
    m = np.empty((P, RPP), np.int64)
    off = 0
    for a in chunks:
        for j in range(a):
            m[:, off + j] = off * P + np.arange(P) * a + j
        off += a
    return m


_XMAP = _row_map(XCHUNKS)
_YMAP = _row_map(YCHUNKS)


def kernel(**inputs) -> np.ndarray:
    x = np.asarray(inputs["x"], dtype=np.float32)
    y = np.asarray(inputs["y"], dtype=np.float32)
    res = _run(x, y, trace=False)
    s = 0.0
    for r in res.results:
        o = r["out"].astype(np.float64)
        sx = np.empty(T); sx[_XMAP.ravel()] = o[:, :RPP].ravel()
        # device col 16 = y rows per YMAP col 15; cols 17.. = YMAP cols 0-14
        sy = np.empty(T)
        sy[_YMAP[:, :RPP - 1].ravel()] = o[:, RPP + 1:].ravel()
        sy[_YMAP[:, RPP - 1]] = o[:, RPP]
        s += (sx * sy).sum()
    return np.array(-s / (B * C * C), dtype=np.float32)
